# revision 63
# baseline (speedup 1.0000x reference)
"""Trainium2 Bass kernel for nn_CFGATLayer (masked graph-attention layer).

Math (per batch b):
  Q = x @ W_q; K = x @ W_k; V = x @ W_v            # [N, F]
  S = (Q @ K^T) / sqrt(F)                          # [N, N]
  S = where(adj == 0, -1e9, S)
  A = softmax(S, axis=-1)
  out = A @ V                                      # [N, F]

Distribution: batch dim (16) sharded over 8 NeuronCores, 2 batches per core.

Additive-mask pipeline (the key restructure vs the v1/STT kernel, 204us ->
169us): W_q is pre-scaled by scale/BIG (BIG=4096) so the PE S matmul lands
t0 = s*scale/BIG in PSUM with |t0| < 0.5 guaranteed (|s*scale| < 2048 whp;
Cauchy-Schwarz + gaussian tails put violation probability below 1e-20).
Per 128-row q-tile:
  PE     : t0 = Qt^T.T @ Kt (f32r, 512-col chunks)          -> PSUM
  DVE/ACT: drain t0 to SBUF f32, split by columns (DVE DCOLS as
           tensor_copy, ACT the rest as Copy-activation) — pure copies.
  Pool   : SWDGE RMW DMA adds adj (int32 HBM, cast to f32) onto the SBUF
           tile: t1 = t0 + adj.  Unmasked lanes land in [0.5, 1.5], masked
           in [-0.5, 0.5], so rowmax(t1) is ALWAYS an unmasked lane: the
           masked row-max rides the adjacency DMA for free — no mask
           elementwise pass, no adj SBUF residency, adj bytes paid once.
           (DMA cce add is the only accum op this walrus accepts, and only
           on the gpsimd/SWDGE path, which is also the only casting path.)
  DVE    : negmax = -rowmax(t1)  (tensor_reduce negate=True)
  Pool/DVE: u = (t1 + negmax)*BIG  (bf16, two-scalar tensor_scalar; DVE
           takes XCOLS, Pool the rest).  Unmasked: u = s*scale - m exactly
           (m = masked row max, so the top surviving weight is exp(0));
           masked: u <= -3500 so exp(u) == 0 in bf16.  Numerically
           identical to an explicit -1e9 mask (verified: same rel err).
  PE     : 16x 128x128 bf16 transposes of u -> u^T (psum)
  ACT    : e^T = exp(u^T) psum->SBUF bf16 (doubles as the psum drain)
  PE     : out^T[f, q] += V_aug[k, f].T @ e^T[k, q] per 4-tile group
           (V_aug has a ones column so row F is the denominator Z)
  PE/DVE : strided transpose-back of out^T so each partition holds 4
           consecutive output rows (1KB-contiguous stores), reciprocal of
           Z, scale, store.

Schedule: a software pipeline over 2-tile pairs with per-stage stagger
  i: S+drains | i-1: adjacency RMW (per tile; leads the Pool queue) |
  i-2: rowmaxes | i-3: subtracts | i-4: transposes+exps |
  i-5: PV + group-finish part 1 | i-6: 1/Z+store (part 2)
so every cross-engine dependency is at least one iteration old when an
in-order engine queue reaches its consumer — queues never park on the
in-flight RMW or on same-iteration producers.  t_p is 6 deep for slot
slack; the last 2 pairs drain fully on DVE because ACT still owes the
staggered exp backlog in the tail.  x is loaded with partition p holding
4 consecutive rows per 512-row block (1KB DMA elements); an ACT copy
reorders the transposed block so Q^T/K^T columns come out natural.  QKV
setup is phase-ordered (all K^T first, then the first Q^T chunk) so the
main pipeline starts while the rest of batch 0's Q^T/V and all of batch
1's QKV stream in behind it; 16 warm-up transposes ramp the PE p-state
during the x loads.

This compiler build accepts only one semaphore-wait command per
instruction; _split_excess_waits() legalizes the BIR by hoisting excess
waits onto EventSemaphore instructions (same engine => same sequencer
order => identical semantics).

Cost-model engine budgets per core (32 q-tiles): DVE ~123 (drain share +
rowmax + sub share + QKV drains), Pool ~123 (SWDGE desc-gen + subtract),
ACT ~122 (drain share + exp + V drains), PE ~95, DMA ~99 (adj stream
93.2 is the floor); span 168.7us = ~20 head + steady + ~22 tail (ACT
exp backlog).
Dead ends, verified empirically on this walrus build: all custom/table
DVE ops and InstTensorTensorReduce ("ISA wrong length"), Pool two-tensor
ops and Pool STT ("engine check failed (Pool)"), Pool reads of PSUM
(BIR verifier), DMA cce mult ("does not support mult with Copy mode"),
DMA to PSUM, f32r operands not written by rounding producers (BIR
verifier), and approximate/sampled softmax stabilizers (NaN tail risk at
32K rows).
"""

import sys

import numpy as np

sys.path.insert(0, "/opt/trn_rl_repo")

B, N, F = 16, 2048, 64
NCORES = 8
NB = B // NCORES  # batches per core
P = 128  # partitions / q-tile rows
BIG = 4096.0  # additive-mask scale; |s|*scale/BIG < 0.5 whp

_PATCHED = False


def _split_excess_waits(bir: bytes) -> bytes:
    """This compiler build only accepts one semaphore-wait command per
    instruction; hoist excess waits onto EventSemaphore instructions placed
    immediately before (same engine => same sequencer order => identical
    semantics)."""
    import orjson
    m = orjson.loads(bir)
    for fn in m["functions"]:
        for blk in fn["blocks"]:
            out = []
            for inst in blk["instructions"]:
                si = inst.get("sync_info")
                waits = (si or {}).get("on_wait") or []
                if len(waits) > 1:
                    for i, w in enumerate(waits[:-1]):
                        out.append({
                            "debug": inst.get("debug"),
                            "engine": inst["engine"],
                            "ins": [], "outs": [],
                            "name": f"{inst['name']}_w{i}",
                            "opcode": "EventSemaphore",
                            "sync_info": {"on_update": [], "on_wait": [w]},
                        })
                    si["on_wait"] = waits[-1:]
                out.append(inst)
            blk["instructions"] = out
    return orjson.dumps(m)


def _install_compile_patch():
    global _PATCHED
    if _PATCHED:
        return
    from concourse import bass_utils, bass2jax

    orig = bass_utils.compile_bir_kernel

    def patched(bir_json, tmpdir, neff_name="file.neff"):
        if isinstance(bir_json, str):
            bir_json = bir_json.encode()
        return orig(_split_excess_waits(bir_json), tmpdir, neff_name=neff_name)

    bass_utils.compile_bir_kernel = patched
    bass2jax.compile_bir_kernel = patched
    _PATCHED = True


def build_kernel(tc, out2, x2, adj2, wq, wk, wv, nb, n, f):
    import concourse.bass as bass
    from concourse import mybir
    from concourse.masks import make_identity
    from concourse.tile_rust import add_dep_helper

    nc = tc.nc
    f32 = mybir.dt.float32
    f32r = mybir.dt.float32r
    bf16 = mybir.dt.bfloat16
    nqt = n // P          # q tiles per batch (16)
    nkc = n // P          # key chunks for PV (16)
    W = n // 2            # psum half width (1024)
    GRP = 4               # q-tiles per PV group
    GW = GRP * P          # group width in q rows (512)
    Fa = f + 1            # V augmented with ones column
    NPAIR = nqt // 2      # tile pairs per batch (8)
    scale = 1.0 / np.sqrt(float(f))
    DCOLS = 640           # drain columns taken by DVE (rest on ACT)
    XCOLS = 192 # subtract columns taken by DVE (rest on Pool)

    _pend = []

    def absorb(*aps):
        return

    def dep(mm):
        for l in _pend:
            add_dep_helper(mm.ins, l.ins, sync=False, reason="wait-absorb")
        return mm

    def flush():
        _pend.clear()

    singles_cm = tc.tile_pool(name="singles", bufs=1)
    singles = singles_cm.__enter__()

    ident_f = singles.tile([P, P], f32)
    make_identity(nc, ident_f)
    ident_b = singles.tile([P, P], bf16)
    make_identity(nc, ident_b)

    wq_sb = singles.tile([f, f], f32)
    wk_sb = singles.tile([f, f], f32)
    wv_sb = singles.tile([f, f], f32)
    nc.sync.dma_start(out=wq_sb, in_=wq)
    nc.sync.dma_start(out=wk_sb, in_=wk)
    nc.sync.dma_start(out=wv_sb, in_=wv)
    wq_r = singles.tile([f, f], f32r)
    wk_r = singles.tile([f, f], f32r)
    wv_r = singles.tile([f, f], f32r)
    # fold the softmax scale AND the additive-mask 1/BIG into W_q
    nc.vector.tensor_scalar(out=wq_r, in0=wq_sb, scalar1=scale / BIG,
                            scalar2=None, op0=mybir.AluOpType.mult)
    nc.vector.tensor_copy(wk_r, wk_sb)
    nc.vector.tensor_copy(wv_r, wv_sb)

    # persistent per-batch tensors
    qt_sb = singles.tile([f, nb, n], f32r)   # Q^T per batch (pre-scaled)
    kt_sb = singles.tile([f, nb, n], f32r)
    v_sb = singles.tile([P, nb, nkc, Fa], bf16)  # V (+ones col) by key chunk

    # main-loop SBUF pools are allocated first so their addresses are
    # disjoint from the setup pools (avoids WAR waits on the first drains)
    t_p_cm = tc.tile_pool(name="t_p", bufs=6)
    u_p_cm = tc.tile_pool(name="u_p", bufs=4)
    eT_p_cm = tc.tile_pool(name="eT_p", bufs=2)
    small_cm = tc.tile_pool(name="small", bufs=8)
    res_p_cm = tc.tile_pool(name="res_p", bufs=2)
    t_p = t_p_cm.__enter__()
    u_p = u_p_cm.__enter__()
    eT_p = eT_p_cm.__enter__()
    small = small_cm.__enter__()
    res_p = res_p_cm.__enter__()

    # ---------------- setup: QKV (chunked) ----------------
    # x is loaded so partition p holds rows {t*512 + 4p + j : j<4} of the
    # batch (1KB contiguous elements).  The per-block transposes produce
    # columns in (j, p) order; the psum->sbuf copy writes them back in
    # (p, j) order so xT columns are the natural row order.  Setup psum
    # tiles borrow the main-loop pool tags (uT/o) since PSUM is fully
    # budgeted; batch 1's chunks are injected into early pipeline
    # iterations so the adjacency stream starts while QKV is still
    # being prepared.
    setup_sb_cm = tc.tile_pool(name="setup_sb", bufs=1)
    setup_sb = setup_sb_cm.__enter__()
    x_tiles = {}
    xT_tiles = {}

    s_ps_pool_cm = tc.tile_pool(name="s_ps", bufs=2, space="PSUM")
    uT_ps_pool_cm = tc.tile_pool(name="uT_ps", bufs=2, space="PSUM")
    o_ps_pool_cm = tc.tile_pool(name="o_ps", bufs=2, space="PSUM")
    s_ps_pool = s_ps_pool_cm.__enter__()
    uT_ps_pool = uT_ps_pool_cm.__enter__()
    o_ps_pool = o_ps_pool_cm.__enter__()

    def setup_x(b):
        x_sb = setup_sb.tile([P, 4, 4, f], f32, tag="x")
        nc.sync.dma_start(
            out=x_sb,
            in_=x2[b].rearrange("(t p j) f -> p t (j f)", p=P, j=4),
        )
        x_tiles[b] = x_sb
        xT_sb = setup_sb.tile([f, 4, P, 4], f32r, tag="xT")
        xT_tiles[b] = xT_sb

    def setup_xt(b, t):
        x_sb = x_tiles[b]
        xT_sb = xT_tiles[b]
        xT_ps = uT_ps_pool.tile([f, 4, P], f32, tag="uT")
        for j in range(4):
            nc.tensor.transpose(xT_ps[:, j, :], x_sb[:, t, j, :], ident_f)
        # reorder (j, p) -> (p, j) while draining
        nc.scalar.copy(xT_sb[:, t].rearrange("f p j -> f j p"), xT_ps)

    def setup_qk(b, t, which):
        xT_sb = xT_tiles[b]
        xT_c = xT_sb[:, t].rearrange("f p j -> f (p j)")
        w_r, dst = ((wq_r, qt_sb), (wk_r, kt_sb))[which]
        qk_ps = uT_ps_pool.tile([f, 512], f32, tag="uT")
        nc.tensor.matmul(qk_ps, lhsT=w_r, rhs=xT_c, start=True, stop=True)
        nc.vector.tensor_copy(dst[:, b, t * 512:(t + 1) * 512], qk_ps)

    def setup_v(b, t):
        xT_sb = xT_tiles[b]
        xT_c = xT_sb[:, t].rearrange("f p j -> f (p j)")
        v_ps = o_ps_pool.tile([P, 4, f], f32, tag="o")
        for kc in range(4):
            nc.tensor.matmul(
                v_ps[:, kc, :],
                lhsT=xT_c[:, kc * P:(kc + 1) * P],
                rhs=wv_r, start=True, stop=True,
            )
        nc.scalar.activation(
            out=v_sb[:, b, 4 * t:4 * t + 4, 0:f], in_=v_ps,
            func=mybir.ActivationFunctionType.Copy, bias=0.0, scale=1.0)

    def setup_chunk(b, phase):
        # one phase per pipeline iteration for the deferred batch
        if phase == 0:
            for t in range(4):
                setup_xt(b, t)
        elif phase == 1:
            for t in range(4):
                setup_qk(b, t, 0)
        elif phase == 2:
            for t in range(4):
                setup_qk(b, t, 1)
        else:
            for t in range(4):
                setup_v(b, t)

    # softmax-denominator ones column (constant; must precede every PV)
    nc.vector.memset(v_sb[:, :, :, f:Fa], 1.0)
    setup_x(0)
    if nb > 1:
        setup_x(1)
    # warm the PE pstate while the x loads are in flight
    for wrm in range(16):
        warm_ps = uT_ps_pool.tile([P, P], bf16, tag="uT")
        nc.tensor.transpose(warm_ps, ident_b, ident_b)
    # minimal prefix for pair 0: all of K^T plus the first q chunk; the
    # rest of batch 0's QKV streams in behind the first S matmuls.
    for t in range(4):
        setup_xt(0, t)
    for t in range(4):
        setup_qk(0, t, 1)
    setup_qk(0, 0, 0)

    # ---------------- main loop ----------------
    if True:
        warm = small.tile([P, 1], f32, tag="dsc")
        nc.vector.memset(warm, 0.0)
        warm2 = small.tile([P, 1], f32, tag="dsc")
        nc.scalar.activation(out=warm2, in_=warm,
                             func=mybir.ActivationFunctionType.Exp)

        prev_exp = [None, None]   # last exp dest slice per half (ACT ticks)
        prev_t = [None, None]     # s_ps slot chase (per half)

        def front_tile(b, pair, ti, t2, dve_only=False):
            """S matmuls + split drain for one tile of the pair.  In the
            pipeline tail ACT is the bottleneck (it still owes the staggered
            exps), so the last pairs drain fully on DVE instead."""
            qi = 2 * pair + ti
            for h in range(2):
                s_ps = s_ps_pool.tile([P, W], f32, tag="s")
                for j in range(W // 512):
                    nc.tensor.matmul(
                        s_ps[:, j * 512:(j + 1) * 512],
                        lhsT=qt_sb[:, b, qi * P:(qi + 1) * P],
                        rhs=kt_sb[:, b,
                                  h * W + j * 512:h * W + (j + 1) * 512],
                        start=True, stop=True,
                    )
                if h == 0:
                    if dve_only:
                        nc.vector.tensor_copy(t2[:, ti, 0:W], s_ps)
                    else:
                        nc.vector.tensor_copy(
                            t2[:, ti, 0:DCOLS], s_ps[:, 0:DCOLS])
                        nc.scalar.activation(
                            out=t2[:, ti, DCOLS:W], in_=s_ps[:, DCOLS:W],
                            func=mybir.ActivationFunctionType.Copy,
                            bias=0.0, scale=1.0)
                else:
                    if dve_only:
                        nc.vector.tensor_copy(t2[:, ti, W:n], s_ps)
                    else:
                        nc.scalar.activation(
                            out=t2[:, ti, W:n], in_=s_ps,
                            func=mybir.ActivationFunctionType.Copy,
                            bias=0.0, scale=1.0)

        def emit_rmw_tile(b, pair, ti, t2):
            qi = 2 * pair + ti
            nc.gpsimd.dma_start(
                out=t2[:, ti, :],
                in_=adj2[b, qi * P:(qi + 1) * P, :],
                accum_op=mybir.AluOpType.add,
            )

        def emit_rmw(b, pair, t2):
            # masked-max mask: t1 = t0 + adj via casting RMW DMAs (SWDGE);
            # one DMA per tile so each rowmax waits only on its own half.
            for ti in range(2):
                emit_rmw_tile(b, pair, ti, t2)

        def emit_reduces(b, pair, t2):
            """negated masked row-max for both tiles of the pair."""
            nms = []
            for ti in range(2):
                negmax = small.tile([P, 1], f32, tag="m")
                nc.vector.tensor_reduce(
                    out=negmax, in_=t2[:, ti, :],
                    axis=mybir.AxisListType.X,
                    op=mybir.AluOpType.max, negate=True,
                )
                nms.append(negmax)
            return nms

        def back_sub(b, pair, t2, nms, tail=False):
            """u = (t1 + negmax) * BIG for both tiles (bf16, Pool + DVE).
            In the tail the two tiles go to DVE and Pool whole, in
            parallel, to shorten the drain-out critical chain."""
            uts = []
            for ti in range(2):
                u_t = u_p.tile([P, n], bf16, tag="u")
                xc = n if (tail and ti == 0) else (0 if tail else XCOLS)
                if xc:
                    nc.vector.tensor_scalar(
                        out=u_t[:, 0:xc], in0=t2[:, ti, 0:xc],
                        scalar1=nms[ti], scalar2=BIG,
                        op0=mybir.AluOpType.add, op1=mybir.AluOpType.mult,
                    )
                if xc < n:
                    nc.gpsimd.tensor_scalar(
                        out=u_t[:, xc:n], in0=t2[:, ti, xc:n],
                        scalar1=nms[ti], scalar2=BIG,
                        op0=mybir.AluOpType.add, op1=mybir.AluOpType.mult,
                    )
                uts.append(u_t)
            return uts

        def back_xe(b, pair, ti, u_t, eT_sb):
            """transpose + exp for one tile."""
            qi = 2 * pair + ti
            g = qi % GRP
            for hh in range(2):
                uT_ps = uT_ps_pool.tile([P, (nkc // 2) * P], bf16, tag="uT")
                for j8 in range(nkc // 2):
                    j = hh * (nkc // 2) + j8
                    nc.tensor.transpose(
                        uT_ps[:, j8 * P:(j8 + 1) * P],
                        u_t[:, j * P:(j + 1) * P],
                        ident_b,
                    )
                exp_dst = eT_sb[:, hh * (nkc // 2):(hh + 1) * (nkc // 2),
                                g * P:(g + 1) * P]
                nc.scalar.activation(
                    out=exp_dst,
                    in_=uT_ps.rearrange("p (j q) -> p j q", q=P),
                    func=mybir.ActivationFunctionType.Exp,
                )

        def pv_half(b, pair, eT_sb, oT_ps):
            """PV over this pair's 256 q-columns of the group."""
            c0 = (pair % 2) * 2 * P
            for j in range(nkc):
                nc.tensor.matmul(
                    oT_ps[:, c0:c0 + 2 * P],
                    lhsT=v_sb[:, b, j, :],
                    rhs=eT_sb[:, j, c0:c0 + 2 * P],
                    start=(j == 0), stop=(j == nkc - 1),
                )

        def finish_a(b, pair, oT_ps):
            """oT drain + strided transpose-back (group part 1)."""
            oT_sb = res_p.tile([Fa, GW], f32, tag="oT")
            nc.scalar.copy(oT_sb, oT_ps)
            oT_v = oT_sb.rearrange("f (p j) -> f j p", j=GRP)
            res4 = o_ps_pool.tile([P, GRP, Fa], f32, tag="o")
            for j in range(GRP):
                nc.tensor.transpose(
                    res4[:, j, :], oT_v[:, j, :], ident_f[0:Fa, 0:Fa],
                )
            return res4

        def finish_b(b, pair, res4):
            """1/Z scale + store (group part 2, one iteration later)."""
            qi = 2 * pair + 1
            r4 = small.tile([P, GRP], f32, tag="r4")
            nc.vector.reciprocal(r4, res4[:, :, f])
            res_sb = res_p.tile([P, GRP, f], f32, tag="res")
            for j in range(GRP):
                nc.vector.tensor_scalar(
                    out=res_sb[:, j, :], in0=res4[:, j, 0:f],
                    scalar1=r4[:, j:j + 1], scalar2=None,
                    op0=mybir.AluOpType.mult,
                )
            q0 = (qi - (GRP - 1)) * P
            nc.sync.dma_start(
                out=out2[b, q0:q0 + GW, :].rearrange(
                    "(p j) f -> p (j f)", p=P),
                in_=res_sb,
            )

        # Fine-grained 6-stage software pipeline (one iteration per tile
        # pair).  Stage offsets ensure every cross-engine dependency was
        # produced >= 1 iteration before an in-order engine queue reaches
        # its consumer, and PE always has ready work (transposes of i-4)
        # queued between the two S-matmul bursts of iteration i:
        #   i: S+drains | i-4: sub/transpose/exp | i-1: adjacency RMW |
        #   i-3: rowmaxes | i-5: half-PV (+ group finish on odd pairs)
        work = [(b, pair) for b in range(nb) for pair in range(NPAIR)]
        NW = len(work)
        tiles, maxes, eTs, oTs, us, res4s = {}, {}, {}, {}, {}, {}
        for i in range(NW + 6):
            if i < NW:
                b, pair = work[i]
                t2_new = t_p.tile([P, 2, n], f32, tag="t")
                tiles[i] = t2_new
                front_tile(b, pair, 0, tiles[i], dve_only=(i >= NW - 2))
                if i < 2:
                    emit_rmw_tile(b, pair, 0, tiles[i])
            if 3 <= i < NW + 1:
                b, pair = work[i - 1]
                emit_rmw(b, pair, tiles[i - 1])
            if 3 <= i < NW + 3:
                b, pair = work[i - 3]
                us[i - 3] = back_sub(b, pair, tiles[i - 3], maxes.pop(i - 3),
                                     tail=(i - 3 >= NW - 2))
            if i < NW:
                b, pair = work[i]
                front_tile(b, pair, 1, tiles[i], dve_only=(i >= NW - 2))
                if i < 2:
                    emit_rmw_tile(b, pair, 1, tiles[i])
            if 4 <= i < NW + 4:
                b, pair = work[i - 4]
                if pair % 2 == 0:
                    eT_new = eT_p.tile([P, nkc, GW], bf16, tag="eT")
                    eTs[i - 4] = eT_new
                else:
                    eTs[i - 4] = eTs[i - 5]
                back_xe(b, pair, 0, us[i - 4][0], eTs[i - 4])
                back_xe(b, pair, 1, us[i - 4][1], eTs[i - 4])
                us.pop(i - 4)
            if 2 <= i < NW + 2:
                b, pair = work[i - 2]
                maxes[i - 2] = emit_reduces(b, pair, tiles[i - 2])
            if 6 <= i < NW + 6 and (i - 6) in res4s:
                fb, fpair, fres4 = res4s.pop(i - 6)
                finish_b(fb, fpair, fres4)
            if 5 <= i < NW + 5:
                b, pair = work[i - 5]
                if pair % 2 == 0:
                    oT_new = o_ps_pool.tile([Fa, GW], f32, tag="o")
                    oTs[i - 5] = oT_new
                else:
                    oTs[i - 5] = oTs[i - 6]
                pv_half(b, pair, eTs[i - 5], oTs[i - 5])
                if pair % 2 == 1:
                    res4s[i - 5] = (b, pair, finish_a(b, pair, oTs[i - 5]))
                    for k in (i - 5, i - 6):
                        eTs.pop(k, None)
                        oTs.pop(k, None)
            if 5 <= i < NW + 5:
                tiles.pop(i - 5, None)
            if i == 0:
                for t in range(1, 4):
                    setup_qk(0, t, 0)
            elif i == 1:
                for t in range(4):
                    setup_v(0, t)
            elif 2 <= i <= 5 and nb > 1:
                setup_chunk(1, i - 2)

    for cm in (o_ps_pool_cm, uT_ps_pool_cm, s_ps_pool_cm, setup_sb_cm,
               res_p_cm, small_cm, eT_p_cm, u_p_cm, t_p_cm):
        cm.__exit__(None, None, None)
    singles_cm.__exit__(None, None, None)


def build_bass(nb=NB, n=N, f=F, num_devices=NCORES):
    import concourse.bass as bass
    import concourse.tile as tile
    from concourse import mybir

    nc = bass.Bass(
        "TRN2", target_bir_lowering=False, debug=False, num_devices=num_devices
    )
    x2 = nc.dram_tensor("x2", [nb, n, f], mybir.dt.float32,
                        kind="ExternalInput").ap()
    adj2 = nc.dram_tensor("adj2", [nb, n, n], mybir.dt.int32,
                          kind="ExternalInput").ap()
    wq = nc.dram_tensor("wq", [f, f], mybir.dt.float32, kind="ExternalInput").ap()
    wk = nc.dram_tensor("wk", [f, f], mybir.dt.float32, kind="ExternalInput").ap()
    wv = nc.dram_tensor("wv", [f, f], mybir.dt.float32, kind="ExternalInput").ap()
    out2 = nc.dram_tensor("out2", [nb, n, f], mybir.dt.float32,
                          kind="ExternalOutput").ap()
    with tile.TileContext(nc) as tc:
        build_kernel(tc, out2, x2, adj2, wq, wk, wv, nb=nb, n=n, f=f)
    return nc


_cached_nc = None


def kernel(x, adj, W_q, W_k, W_v, _trace=False):
    global _cached_nc
    _install_compile_patch()
    from concourse import bass_utils

    if _cached_nc is None:
        _cached_nc = build_bass()
    nc = _cached_nc

    x = np.ascontiguousarray(np.asarray(x, dtype=np.float32))
    adj = np.ascontiguousarray(np.asarray(adj, dtype=np.int32))
    wq = np.ascontiguousarray(np.asarray(W_q, dtype=np.float32))
    wk = np.ascontiguousarray(np.asarray(W_k, dtype=np.float32))
    wv = np.ascontiguousarray(np.asarray(W_v, dtype=np.float32))

    in_maps = []
    for c in range(NCORES):
        in_maps.append({
            "x2": x[c * NB:(c + 1) * NB],
            "adj2": adj[c * NB:(c + 1) * NB],
            "wq": wq, "wk": wk, "wv": wv,
        })
    res = bass_utils.run_bass_kernel_spmd(
        nc, in_maps, core_ids=list(range(NCORES)), trace=_trace,
    )
    out = np.concatenate([r["out2"] for r in res.results], axis=0)
    if _trace:
        kernel._last_results = res
    return out.reshape(B, N, F)


# revision 67
# speedup vs baseline: 1.0066x; 1.0066x over previous
"""Trainium2 Bass kernel for nn_CFGATLayer (masked graph-attention layer).

Math (per batch b):
  Q = x @ W_q; K = x @ W_k; V = x @ W_v            # [N, F]
  S = (Q @ K^T) / sqrt(F)                          # [N, N]
  S = where(adj == 0, -1e9, S)
  A = softmax(S, axis=-1)
  out = A @ V                                      # [N, F]

Distribution: batch dim (16) sharded over 8 NeuronCores, 2 batches per core.

Additive-mask pipeline (the key restructure vs the v1/STT kernel, 204us ->
169us): W_q is pre-scaled by scale/BIG (BIG=4096) so the PE S matmul lands
t0 = s*scale/BIG in PSUM with |t0| < 0.5 guaranteed (|s*scale| < 2048 whp;
Cauchy-Schwarz + gaussian tails put violation probability below 1e-20).
Per 128-row q-tile:
  PE     : t0 = Qt^T.T @ Kt (f32r, 512-col chunks)          -> PSUM
  DVE/ACT: drain t0 to SBUF f32, split by columns (DVE DCOLS as
           tensor_copy, ACT the rest as Copy-activation) — pure copies.
  Pool   : SWDGE RMW DMA adds adj (int32 HBM, cast to f32) onto the SBUF
           tile: t1 = t0 + adj.  Unmasked lanes land in [0.5, 1.5], masked
           in [-0.5, 0.5], so rowmax(t1) is ALWAYS an unmasked lane: the
           masked row-max rides the adjacency DMA for free — no mask
           elementwise pass, no adj SBUF residency, adj bytes paid once.
           (DMA cce add is the only accum op this walrus accepts, and only
           on the gpsimd/SWDGE path, which is also the only casting path.)
  DVE    : negmax = -rowmax(t1)  (tensor_reduce negate=True)
  Pool/DVE: u = (t1 + negmax)*BIG  (bf16, two-scalar tensor_scalar; DVE
           takes XCOLS, Pool the rest).  Unmasked: u = s*scale - m exactly
           (m = masked row max, so the top surviving weight is exp(0));
           masked: u <= -3500 so exp(u) == 0 in bf16.  Numerically
           identical to an explicit -1e9 mask (verified: same rel err).
  PE     : 16x 128x128 bf16 transposes of u -> u^T (psum)
  ACT    : e^T = exp(u^T) psum->SBUF bf16 (doubles as the psum drain)
  PE     : out^T[f, q] += V_aug[k, f].T @ e^T[k, q] per 4-tile group
           (V_aug has a ones column so row F is the denominator Z)
  PE/DVE : strided transpose-back of out^T so each partition holds 4
           consecutive output rows (1KB-contiguous stores), reciprocal of
           Z, scale, store.

Schedule: a software pipeline over 2-tile pairs with per-stage stagger
  i: S+drains | i-1: adjacency RMW (per tile; leads the Pool queue) |
  i-2: rowmaxes | i-3: subtracts | i-4: transposes+exps |
  i-5: PV + group-finish part 1 | i-6: 1/Z+store (part 2)
so every cross-engine dependency is at least one iteration old when an
in-order engine queue reaches its consumer — queues never park on the
in-flight RMW or on same-iteration producers.  t_p is 6 deep for slot
slack; the last 2 pairs drain fully on DVE because ACT still owes the
staggered exp backlog in the tail.  x is loaded with partition p holding
4 consecutive rows per 512-row block (1KB DMA elements); an ACT copy
reorders the transposed block so Q^T/K^T columns come out natural.  QKV
setup is phase-ordered (all K^T first, then the first Q^T chunk) so the
main pipeline starts while the rest of batch 0's Q^T/V and all of batch
1's QKV stream in behind it; 12 warm-up transposes ramp the PE p-state
during the (per-block-chunked) x loads.

This compiler build accepts only one semaphore-wait command per
instruction; _split_excess_waits() legalizes the BIR by hoisting excess
waits onto EventSemaphore instructions (same engine => same sequencer
order => identical semantics).

Cost-model engine budgets per core (32 q-tiles): DVE ~123 (drain share +
rowmax + sub share + QKV drains), Pool ~123 (SWDGE desc-gen + subtract),
ACT ~122 (drain share + exp + V drains), PE ~95, DMA ~99 (adj stream
93.2 is the floor); span 167.6us = ~19 head + steady + ~22 tail (ACT
exp backlog).
Dead ends, verified empirically on this walrus build: all custom/table
DVE ops and InstTensorTensorReduce ("ISA wrong length"), Pool two-tensor
ops and Pool STT ("engine check failed (Pool)"), Pool reads of PSUM
(BIR verifier), DMA cce mult ("does not support mult with Copy mode"),
DMA to PSUM, f32r operands not written by rounding producers (BIR
verifier), and approximate/sampled softmax stabilizers (NaN tail risk at
32K rows).
"""

import sys

import numpy as np

sys.path.insert(0, "/opt/trn_rl_repo")

B, N, F = 16, 2048, 64
NCORES = 8
NB = B // NCORES  # batches per core
P = 128  # partitions / q-tile rows
BIG = 4096.0  # additive-mask scale; |s|*scale/BIG < 0.5 whp

_PATCHED = False


def _split_excess_waits(bir: bytes) -> bytes:
    """This compiler build only accepts one semaphore-wait command per
    instruction; hoist excess waits onto EventSemaphore instructions placed
    immediately before (same engine => same sequencer order => identical
    semantics)."""
    import orjson
    m = orjson.loads(bir)
    for fn in m["functions"]:
        for blk in fn["blocks"]:
            out = []
            for inst in blk["instructions"]:
                si = inst.get("sync_info")
                waits = (si or {}).get("on_wait") or []
                if len(waits) > 1:
                    for i, w in enumerate(waits[:-1]):
                        out.append({
                            "debug": inst.get("debug"),
                            "engine": inst["engine"],
                            "ins": [], "outs": [],
                            "name": f"{inst['name']}_w{i}",
                            "opcode": "EventSemaphore",
                            "sync_info": {"on_update": [], "on_wait": [w]},
                        })
                    si["on_wait"] = waits[-1:]
                out.append(inst)
            blk["instructions"] = out
    return orjson.dumps(m)


def _install_compile_patch():
    global _PATCHED
    if _PATCHED:
        return
    from concourse import bass_utils, bass2jax

    orig = bass_utils.compile_bir_kernel

    def patched(bir_json, tmpdir, neff_name="file.neff"):
        if isinstance(bir_json, str):
            bir_json = bir_json.encode()
        return orig(_split_excess_waits(bir_json), tmpdir, neff_name=neff_name)

    bass_utils.compile_bir_kernel = patched
    bass2jax.compile_bir_kernel = patched
    _PATCHED = True


def build_kernel(tc, out2, x2, adj2, wq, wk, wv, nb, n, f):
    import concourse.bass as bass
    from concourse import mybir
    from concourse.masks import make_identity
    from concourse.tile_rust import add_dep_helper

    nc = tc.nc
    f32 = mybir.dt.float32
    f32r = mybir.dt.float32r
    bf16 = mybir.dt.bfloat16
    nqt = n // P          # q tiles per batch (16)
    nkc = n // P          # key chunks for PV (16)
    W = n // 2            # psum half width (1024)
    GRP = 4               # q-tiles per PV group
    GW = GRP * P          # group width in q rows (512)
    Fa = f + 1            # V augmented with ones column
    NPAIR = nqt // 2      # tile pairs per batch (8)
    scale = 1.0 / np.sqrt(float(f))
    DCOLS = 640           # drain columns taken by DVE (rest on ACT)
    XCOLS = 192 # subtract columns taken by DVE (rest on Pool)

    _pend = []

    def absorb(*aps):
        return

    def dep(mm):
        for l in _pend:
            add_dep_helper(mm.ins, l.ins, sync=False, reason="wait-absorb")
        return mm

    def flush():
        _pend.clear()

    singles_cm = tc.tile_pool(name="singles", bufs=1)
    singles = singles_cm.__enter__()

    ident_f = singles.tile([P, P], f32)
    make_identity(nc, ident_f)
    ident_b = singles.tile([P, P], bf16)
    make_identity(nc, ident_b)

    wq_sb = singles.tile([f, f], f32)
    wk_sb = singles.tile([f, f], f32)
    wv_sb = singles.tile([f, f], f32)
    nc.sync.dma_start(out=wq_sb, in_=wq)
    nc.sync.dma_start(out=wk_sb, in_=wk)
    nc.sync.dma_start(out=wv_sb, in_=wv)
    wq_r = singles.tile([f, f], f32r)
    wk_r = singles.tile([f, f], f32r)
    wv_r = singles.tile([f, f], f32r)
    # fold the softmax scale AND the additive-mask 1/BIG into W_q
    nc.vector.tensor_scalar(out=wq_r, in0=wq_sb, scalar1=scale / BIG,
                            scalar2=None, op0=mybir.AluOpType.mult)
    nc.vector.tensor_copy(wk_r, wk_sb)
    nc.vector.tensor_copy(wv_r, wv_sb)

    # persistent per-batch tensors
    qt_sb = singles.tile([f, nb, n], f32r)   # Q^T per batch (pre-scaled)
    kt_sb = singles.tile([f, nb, n], f32r)
    v_sb = singles.tile([P, nb, nkc, Fa], bf16)  # V (+ones col) by key chunk

    # main-loop SBUF pools are allocated first so their addresses are
    # disjoint from the setup pools (avoids WAR waits on the first drains)
    t_p_cm = tc.tile_pool(name="t_p", bufs=6)
    u_p_cm = tc.tile_pool(name="u_p", bufs=4)
    eT_p_cm = tc.tile_pool(name="eT_p", bufs=2)
    small_cm = tc.tile_pool(name="small", bufs=8)
    res_p_cm = tc.tile_pool(name="res_p", bufs=2)
    t_p = t_p_cm.__enter__()
    u_p = u_p_cm.__enter__()
    eT_p = eT_p_cm.__enter__()
    small = small_cm.__enter__()
    res_p = res_p_cm.__enter__()

    # ---------------- setup: QKV (chunked) ----------------
    # x is loaded so partition p holds rows {t*512 + 4p + j : j<4} of the
    # batch (1KB contiguous elements).  The per-block transposes produce
    # columns in (j, p) order; the psum->sbuf copy writes them back in
    # (p, j) order so xT columns are the natural row order.  Setup psum
    # tiles borrow the main-loop pool tags (uT/o) since PSUM is fully
    # budgeted; batch 1's chunks are injected into early pipeline
    # iterations so the adjacency stream starts while QKV is still
    # being prepared.
    setup_sb_cm = tc.tile_pool(name="setup_sb", bufs=1)
    setup_sb = setup_sb_cm.__enter__()
    x_tiles = {}
    xT_tiles = {}

    s_ps_pool_cm = tc.tile_pool(name="s_ps", bufs=2, space="PSUM")
    uT_ps_pool_cm = tc.tile_pool(name="uT_ps", bufs=2, space="PSUM")
    o_ps_pool_cm = tc.tile_pool(name="o_ps", bufs=2, space="PSUM")
    s_ps_pool = s_ps_pool_cm.__enter__()
    uT_ps_pool = uT_ps_pool_cm.__enter__()
    o_ps_pool = o_ps_pool_cm.__enter__()

    def setup_x(b):
        # per-512-row-block loads so the first transpose only waits on a
        # quarter of the batch's x
        x_sb = setup_sb.tile([P, 4, 4, f], f32, tag="x")
        xv = x2[b].rearrange("(t p j) f -> p t (j f)", p=P, j=4)
        for t in range(4):
            nc.sync.dma_start(out=x_sb[:, t], in_=xv[:, t])
        x_tiles[b] = x_sb
        xT_sb = setup_sb.tile([f, 4, P, 4], f32r, tag="xT")
        xT_tiles[b] = xT_sb

    def setup_xt(b, t):
        x_sb = x_tiles[b]
        xT_sb = xT_tiles[b]
        xT_ps = uT_ps_pool.tile([f, 4, P], f32, tag="uT")
        for j in range(4):
            nc.tensor.transpose(xT_ps[:, j, :], x_sb[:, t, j, :], ident_f)
        # reorder (j, p) -> (p, j) while draining
        nc.scalar.copy(xT_sb[:, t].rearrange("f p j -> f j p"), xT_ps)

    def setup_qk(b, t, which):
        xT_sb = xT_tiles[b]
        xT_c = xT_sb[:, t].rearrange("f p j -> f (p j)")
        w_r, dst = ((wq_r, qt_sb), (wk_r, kt_sb))[which]
        qk_ps = uT_ps_pool.tile([f, 512], f32, tag="uT")
        nc.tensor.matmul(qk_ps, lhsT=w_r, rhs=xT_c, start=True, stop=True)
        nc.vector.tensor_copy(dst[:, b, t * 512:(t + 1) * 512], qk_ps)

    def setup_v(b, t):
        xT_sb = xT_tiles[b]
        xT_c = xT_sb[:, t].rearrange("f p j -> f (p j)")
        v_ps = o_ps_pool.tile([P, 4, f], f32, tag="o")
        for kc in range(4):
            nc.tensor.matmul(
                v_ps[:, kc, :],
                lhsT=xT_c[:, kc * P:(kc + 1) * P],
                rhs=wv_r, start=True, stop=True,
            )
        nc.scalar.activation(
            out=v_sb[:, b, 4 * t:4 * t + 4, 0:f], in_=v_ps,
            func=mybir.ActivationFunctionType.Copy, bias=0.0, scale=1.0)

    def setup_chunk(b, phase):
        # one phase per pipeline iteration for the deferred batch
        if phase == 0:
            for t in range(4):
                setup_xt(b, t)
        elif phase == 1:
            for t in range(4):
                setup_qk(b, t, 0)
        elif phase == 2:
            for t in range(4):
                setup_qk(b, t, 1)
        else:
            for t in range(4):
                setup_v(b, t)

    # softmax-denominator ones column (constant; must precede every PV)
    nc.vector.memset(v_sb[:, :, :, f:Fa], 1.0)
    setup_x(0)
    if nb > 1:
        setup_x(1)
    # warm the PE pstate while the x loads are in flight
    for wrm in range(12):
        warm_ps = uT_ps_pool.tile([P, P], bf16, tag="uT")
        nc.tensor.transpose(warm_ps, ident_b, ident_b)
    # minimal prefix for pair 0: all of K^T plus the first q chunk; the
    # rest of batch 0's QKV streams in behind the first S matmuls.
    for t in range(4):
        setup_xt(0, t)
    for t in range(4):
        setup_qk(0, t, 1)
    setup_qk(0, 0, 0)

    # ---------------- main loop ----------------
    if True:
        warm = small.tile([P, 1], f32, tag="dsc")
        nc.vector.memset(warm, 0.0)
        warm2 = small.tile([P, 1], f32, tag="dsc")
        nc.scalar.activation(out=warm2, in_=warm,
                             func=mybir.ActivationFunctionType.Exp)

        prev_exp = [None, None]   # last exp dest slice per half (ACT ticks)
        prev_t = [None, None]     # s_ps slot chase (per half)

        def front_tile(b, pair, ti, t2, dve_only=False):
            """S matmuls + split drain for one tile of the pair.  In the
            pipeline tail ACT is the bottleneck (it still owes the staggered
            exps), so the last pairs drain fully on DVE instead."""
            qi = 2 * pair + ti
            for h in range(2):
                s_ps = s_ps_pool.tile([P, W], f32, tag="s")
                for j in range(W // 512):
                    nc.tensor.matmul(
                        s_ps[:, j * 512:(j + 1) * 512],
                        lhsT=qt_sb[:, b, qi * P:(qi + 1) * P],
                        rhs=kt_sb[:, b,
                                  h * W + j * 512:h * W + (j + 1) * 512],
                        start=True, stop=True,
                    )
                if h == 0:
                    if dve_only:
                        nc.vector.tensor_copy(t2[:, ti, 0:W], s_ps)
                    else:
                        nc.vector.tensor_copy(
                            t2[:, ti, 0:DCOLS], s_ps[:, 0:DCOLS])
                        nc.scalar.activation(
                            out=t2[:, ti, DCOLS:W], in_=s_ps[:, DCOLS:W],
                            func=mybir.ActivationFunctionType.Copy,
                            bias=0.0, scale=1.0)
                else:
                    if dve_only:
                        nc.vector.tensor_copy(t2[:, ti, W:n], s_ps)
                    else:
                        nc.scalar.activation(
                            out=t2[:, ti, W:n], in_=s_ps,
                            func=mybir.ActivationFunctionType.Copy,
                            bias=0.0, scale=1.0)

        def emit_rmw_tile(b, pair, ti, t2):
            qi = 2 * pair + ti
            nc.gpsimd.dma_start(
                out=t2[:, ti, :],
                in_=adj2[b, qi * P:(qi + 1) * P, :],
                accum_op=mybir.AluOpType.add,
            )

        def emit_rmw(b, pair, t2):
            # masked-max mask: t1 = t0 + adj via casting RMW DMAs (SWDGE);
            # one DMA per tile so each rowmax waits only on its own half.
            for ti in range(2):
                emit_rmw_tile(b, pair, ti, t2)

        def emit_reduces(b, pair, t2):
            """negated masked row-max for both tiles of the pair."""
            nms = []
            for ti in range(2):
                negmax = small.tile([P, 1], f32, tag="m")
                nc.vector.tensor_reduce(
                    out=negmax, in_=t2[:, ti, :],
                    axis=mybir.AxisListType.X,
                    op=mybir.AluOpType.max, negate=True,
                )
                nms.append(negmax)
            return nms

        def back_sub(b, pair, t2, nms, tail=False):
            """u = (t1 + negmax) * BIG for both tiles (bf16, Pool + DVE).
            In the tail the two tiles go to DVE and Pool whole, in
            parallel, to shorten the drain-out critical chain."""
            uts = []
            for ti in range(2):
                u_t = u_p.tile([P, n], bf16, tag="u")
                xc = n if (tail and ti == 0) else (0 if tail else XCOLS)
                if xc:
                    nc.vector.tensor_scalar(
                        out=u_t[:, 0:xc], in0=t2[:, ti, 0:xc],
                        scalar1=nms[ti], scalar2=BIG,
                        op0=mybir.AluOpType.add, op1=mybir.AluOpType.mult,
                    )
                if xc < n:
                    nc.gpsimd.tensor_scalar(
                        out=u_t[:, xc:n], in0=t2[:, ti, xc:n],
                        scalar1=nms[ti], scalar2=BIG,
                        op0=mybir.AluOpType.add, op1=mybir.AluOpType.mult,
                    )
                uts.append(u_t)
            return uts

        def back_xe(b, pair, ti, u_t, eT_sb):
            """transpose + exp for one tile."""
            qi = 2 * pair + ti
            g = qi % GRP
            for hh in range(2):
                uT_ps = uT_ps_pool.tile([P, (nkc // 2) * P], bf16, tag="uT")
                for j8 in range(nkc // 2):
                    j = hh * (nkc // 2) + j8
                    nc.tensor.transpose(
                        uT_ps[:, j8 * P:(j8 + 1) * P],
                        u_t[:, j * P:(j + 1) * P],
                        ident_b,
                    )
                exp_dst = eT_sb[:, hh * (nkc // 2):(hh + 1) * (nkc // 2),
                                g * P:(g + 1) * P]
                nc.scalar.activation(
                    out=exp_dst,
                    in_=uT_ps.rearrange("p (j q) -> p j q", q=P),
                    func=mybir.ActivationFunctionType.Exp,
                )

        def pv_half(b, pair, eT_sb, oT_ps):
            """PV over this pair's 256 q-columns of the group."""
            c0 = (pair % 2) * 2 * P
            for j in range(nkc):
                nc.tensor.matmul(
                    oT_ps[:, c0:c0 + 2 * P],
                    lhsT=v_sb[:, b, j, :],
                    rhs=eT_sb[:, j, c0:c0 + 2 * P],
                    start=(j == 0), stop=(j == nkc - 1),
                )

        def finish_a(b, pair, oT_ps):
            """oT drain + strided transpose-back (group part 1)."""
            oT_sb = res_p.tile([Fa, GW], f32, tag="oT")
            nc.scalar.copy(oT_sb, oT_ps)
            oT_v = oT_sb.rearrange("f (p j) -> f j p", j=GRP)
            res4 = o_ps_pool.tile([P, GRP, Fa], f32, tag="o")
            for j in range(GRP):
                nc.tensor.transpose(
                    res4[:, j, :], oT_v[:, j, :], ident_f[0:Fa, 0:Fa],
                )
            return res4

        def finish_b(b, pair, res4):
            """1/Z scale + store (group part 2, one iteration later)."""
            qi = 2 * pair + 1
            r4 = small.tile([P, GRP], f32, tag="r4")
            nc.vector.reciprocal(r4, res4[:, :, f])
            res_sb = res_p.tile([P, GRP, f], f32, tag="res")
            for j in range(GRP):
                nc.vector.tensor_scalar(
                    out=res_sb[:, j, :], in0=res4[:, j, 0:f],
                    scalar1=r4[:, j:j + 1], scalar2=None,
                    op0=mybir.AluOpType.mult,
                )
            q0 = (qi - (GRP - 1)) * P
            nc.sync.dma_start(
                out=out2[b, q0:q0 + GW, :].rearrange(
                    "(p j) f -> p (j f)", p=P),
                in_=res_sb,
            )

        # Fine-grained 6-stage software pipeline (one iteration per tile
        # pair).  Stage offsets ensure every cross-engine dependency was
        # produced >= 1 iteration before an in-order engine queue reaches
        # its consumer, and PE always has ready work (transposes of i-4)
        # queued between the two S-matmul bursts of iteration i:
        #   i: S+drains | i-4: sub/transpose/exp | i-1: adjacency RMW |
        #   i-3: rowmaxes | i-5: half-PV (+ group finish on odd pairs)
        work = [(b, pair) for b in range(nb) for pair in range(NPAIR)]
        NW = len(work)
        tiles, maxes, eTs, oTs, us, res4s = {}, {}, {}, {}, {}, {}
        for i in range(NW + 6):
            if i < NW:
                b, pair = work[i]
                t2_new = t_p.tile([P, 2, n], f32, tag="t")
                tiles[i] = t2_new
                front_tile(b, pair, 0, tiles[i], dve_only=(i >= NW - 2))
                if i < 2:
                    emit_rmw_tile(b, pair, 0, tiles[i])
            if 3 <= i < NW + 1:
                b, pair = work[i - 1]
                emit_rmw(b, pair, tiles[i - 1])
            if 3 <= i < NW + 3:
                b, pair = work[i - 3]
                us[i - 3] = back_sub(b, pair, tiles[i - 3], maxes.pop(i - 3),
                                     tail=(i - 3 >= NW - 2))
            if i < NW:
                b, pair = work[i]
                front_tile(b, pair, 1, tiles[i], dve_only=(i >= NW - 2))
                if i < 2:
                    emit_rmw_tile(b, pair, 1, tiles[i])
            if 4 <= i < NW + 4:
                b, pair = work[i - 4]
                if pair % 2 == 0:
                    eT_new = eT_p.tile([P, nkc, GW], bf16, tag="eT")
                    eTs[i - 4] = eT_new
                else:
                    eTs[i - 4] = eTs[i - 5]
                back_xe(b, pair, 0, us[i - 4][0], eTs[i - 4])
                back_xe(b, pair, 1, us[i - 4][1], eTs[i - 4])
                us.pop(i - 4)
            if 2 <= i < NW + 2:
                b, pair = work[i - 2]
                maxes[i - 2] = emit_reduces(b, pair, tiles[i - 2])
            if 6 <= i < NW + 6 and (i - 6) in res4s:
                fb, fpair, fres4 = res4s.pop(i - 6)
                finish_b(fb, fpair, fres4)
            if 5 <= i < NW + 5:
                b, pair = work[i - 5]
                if pair % 2 == 0:
                    oT_new = o_ps_pool.tile([Fa, GW], f32, tag="o")
                    oTs[i - 5] = oT_new
                else:
                    oTs[i - 5] = oTs[i - 6]
                pv_half(b, pair, eTs[i - 5], oTs[i - 5])
                if pair % 2 == 1:
                    res4s[i - 5] = (b, pair, finish_a(b, pair, oTs[i - 5]))
                    for k in (i - 5, i - 6):
                        eTs.pop(k, None)
                        oTs.pop(k, None)
            if 5 <= i < NW + 5:
                tiles.pop(i - 5, None)
            if i == 0:
                for t in range(1, 4):
                    setup_qk(0, t, 0)
            elif i == 1:
                for t in range(4):
                    setup_v(0, t)
            elif 2 <= i <= 5 and nb > 1:
                setup_chunk(1, i - 2)

    for cm in (o_ps_pool_cm, uT_ps_pool_cm, s_ps_pool_cm, setup_sb_cm,
               res_p_cm, small_cm, eT_p_cm, u_p_cm, t_p_cm):
        cm.__exit__(None, None, None)
    singles_cm.__exit__(None, None, None)


def build_bass(nb=NB, n=N, f=F, num_devices=NCORES):
    import concourse.bass as bass
    import concourse.tile as tile
    from concourse import mybir

    nc = bass.Bass(
        "TRN2", target_bir_lowering=False, debug=False, num_devices=num_devices
    )
    x2 = nc.dram_tensor("x2", [nb, n, f], mybir.dt.float32,
                        kind="ExternalInput").ap()
    adj2 = nc.dram_tensor("adj2", [nb, n, n], mybir.dt.int32,
                          kind="ExternalInput").ap()
    wq = nc.dram_tensor("wq", [f, f], mybir.dt.float32, kind="ExternalInput").ap()
    wk = nc.dram_tensor("wk", [f, f], mybir.dt.float32, kind="ExternalInput").ap()
    wv = nc.dram_tensor("wv", [f, f], mybir.dt.float32, kind="ExternalInput").ap()
    out2 = nc.dram_tensor("out2", [nb, n, f], mybir.dt.float32,
                          kind="ExternalOutput").ap()
    with tile.TileContext(nc) as tc:
        build_kernel(tc, out2, x2, adj2, wq, wk, wv, nb=nb, n=n, f=f)
    return nc


_cached_nc = None


def kernel(x, adj, W_q, W_k, W_v, _trace=False):
    global _cached_nc
    _install_compile_patch()
    from concourse import bass_utils

    if _cached_nc is None:
        _cached_nc = build_bass()
    nc = _cached_nc

    x = np.ascontiguousarray(np.asarray(x, dtype=np.float32))
    adj = np.ascontiguousarray(np.asarray(adj, dtype=np.int32))
    wq = np.ascontiguousarray(np.asarray(W_q, dtype=np.float32))
    wk = np.ascontiguousarray(np.asarray(W_k, dtype=np.float32))
    wv = np.ascontiguousarray(np.asarray(W_v, dtype=np.float32))

    in_maps = []
    for c in range(NCORES):
        in_maps.append({
            "x2": x[c * NB:(c + 1) * NB],
            "adj2": adj[c * NB:(c + 1) * NB],
            "wq": wq, "wk": wk, "wv": wv,
        })
    res = bass_utils.run_bass_kernel_spmd(
        nc, in_maps, core_ids=list(range(NCORES)), trace=_trace,
    )
    out = np.concatenate([r["out2"] for r in res.results], axis=0)
    if _trace:
        kernel._last_results = res
    return out.reshape(B, N, F)


# revision 70
# speedup vs baseline: 1.0158x; 1.0091x over previous
"""Trainium2 Bass kernel for nn_CFGATLayer (masked graph-attention layer).

Math (per batch b):
  Q = x @ W_q; K = x @ W_k; V = x @ W_v            # [N, F]
  S = (Q @ K^T) / sqrt(F)                          # [N, N]
  S = where(adj == 0, -1e9, S)
  A = softmax(S, axis=-1)
  out = A @ V                                      # [N, F]

Distribution: batch dim (16) sharded over 8 NeuronCores, 2 batches per core.

Additive-mask pipeline (the key restructure vs the v1/STT kernel, 204us ->
169us): W_q is pre-scaled by scale/BIG (BIG=4096) so the PE S matmul lands
t0 = s*scale/BIG in PSUM with |t0| < 0.5 guaranteed (|s*scale| < 2048 whp;
Cauchy-Schwarz + gaussian tails put violation probability below 1e-20).
Per 128-row q-tile:
  PE     : t0 = Qt^T.T @ Kt (f32r, 512-col chunks)          -> PSUM
  DVE/ACT: drain t0 to SBUF f32, split by columns (DVE DCOLS as
           tensor_copy, ACT the rest as Copy-activation) — pure copies.
  Pool   : SWDGE RMW DMA adds adj (int32 HBM, cast to f32) onto the SBUF
           tile: t1 = t0 + adj.  Unmasked lanes land in [0.5, 1.5], masked
           in [-0.5, 0.5], so rowmax(t1) is ALWAYS an unmasked lane: the
           masked row-max rides the adjacency DMA for free — no mask
           elementwise pass, no adj SBUF residency, adj bytes paid once.
           (DMA cce add is the only accum op this walrus accepts, and only
           on the gpsimd/SWDGE path, which is also the only casting path.)
  DVE    : negmax = -rowmax(t1)  (tensor_reduce negate=True)
  Pool/DVE: u = (t1 + negmax)*BIG  (bf16, two-scalar tensor_scalar; DVE
           takes XCOLS, Pool the rest).  Unmasked: u = s*scale - m exactly
           (m = masked row max, so the top surviving weight is exp(0));
           masked: u <= -3500 so exp(u) == 0 in bf16.  Numerically
           identical to an explicit -1e9 mask (verified: same rel err).
  PE     : 16x 128x128 bf16 transposes of u -> u^T (psum)
  ACT    : e^T = exp(u^T) psum->SBUF bf16 (doubles as the psum drain)
  PE     : out^T[f, q] += V_aug[k, f].T @ e^T[k, q] per 4-tile group
           (V_aug has a ones column so row F is the denominator Z)
  PE/DVE : strided transpose-back of out^T so each partition holds 4
           consecutive output rows (1KB-contiguous stores), reciprocal of
           Z, scale, store.

Schedule: a software pipeline over 2-tile pairs with per-stage stagger
  i: S+drains | i-1: adjacency RMW (per tile; leads the Pool queue) |
  i-2: rowmaxes | i-3: subtracts | i-4: transposes+exps |
  i-5: PV + group-finish part 1 | i-6: 1/Z+store (part 2)
so every cross-engine dependency is at least one iteration old when an
in-order engine queue reaches its consumer — queues never park on the
in-flight RMW or on same-iteration producers.  t_p is 6 deep for slot
slack; the last 2 pairs drain fully on DVE because ACT still owes the
staggered exp backlog in the tail.  x is loaded with partition p holding
4 consecutive rows per 512-row block (1KB DMA elements); an ACT copy
reorders the transposed block so Q^T/K^T columns come out natural.  QKV
setup is phase-ordered (all K^T first, then the first Q^T chunk) so the
main pipeline starts while the rest of batch 0's Q^T/V and all of batch
1's QKV stream in behind it; 12 warm-up transposes ramp the PE p-state
during the (per-block-chunked) x loads.

This compiler build accepts only one semaphore-wait command per
instruction; _split_excess_waits() legalizes the BIR by hoisting excess
waits onto EventSemaphore instructions (same engine => same sequencer
order => identical semantics).

Cost-model engine budgets per core (32 q-tiles): DVE ~123 (drain share +
rowmax + sub share + QKV drains), Pool ~123 (SWDGE desc-gen + subtract),
ACT ~122 (drain share + exp + V drains), PE ~95, DMA ~99 (adj stream
93.2 is the floor); span 167.6us = ~19 head + steady + ~22 tail (ACT
exp backlog).
Dead ends, verified empirically on this walrus build: all custom/table
DVE ops and InstTensorTensorReduce ("ISA wrong length"), Pool two-tensor
ops and Pool STT ("engine check failed (Pool)"), Pool reads of PSUM
(BIR verifier), DMA cce mult ("does not support mult with Copy mode"),
DMA to PSUM, f32r operands not written by rounding producers (BIR
verifier), and approximate/sampled softmax stabilizers (NaN tail risk at
32K rows).
"""

import sys

import numpy as np

sys.path.insert(0, "/opt/trn_rl_repo")

B, N, F = 16, 2048, 64
NCORES = 8
NB = B // NCORES  # batches per core
P = 128  # partitions / q-tile rows
BIG = 4096.0  # additive-mask scale; |s|*scale/BIG < 0.5 whp

_PATCHED = False


def _split_excess_waits(bir: bytes) -> bytes:
    """This compiler build only accepts one semaphore-wait command per
    instruction; hoist excess waits onto EventSemaphore instructions placed
    immediately before (same engine => same sequencer order => identical
    semantics)."""
    import orjson
    m = orjson.loads(bir)
    for fn in m["functions"]:
        for blk in fn["blocks"]:
            out = []
            for inst in blk["instructions"]:
                si = inst.get("sync_info")
                waits = (si or {}).get("on_wait") or []
                if len(waits) > 1:
                    for i, w in enumerate(waits[:-1]):
                        out.append({
                            "debug": inst.get("debug"),
                            "engine": inst["engine"],
                            "ins": [], "outs": [],
                            "name": f"{inst['name']}_w{i}",
                            "opcode": "EventSemaphore",
                            "sync_info": {"on_update": [], "on_wait": [w]},
                        })
                    si["on_wait"] = waits[-1:]
                out.append(inst)
            blk["instructions"] = out
    return orjson.dumps(m)


def _install_compile_patch():
    global _PATCHED
    if _PATCHED:
        return
    from concourse import bass_utils, bass2jax

    orig = bass_utils.compile_bir_kernel

    def patched(bir_json, tmpdir, neff_name="file.neff"):
        if isinstance(bir_json, str):
            bir_json = bir_json.encode()
        return orig(_split_excess_waits(bir_json), tmpdir, neff_name=neff_name)

    bass_utils.compile_bir_kernel = patched
    bass2jax.compile_bir_kernel = patched
    _PATCHED = True


def build_kernel(tc, out2, x2, adj2, wq, wk, wv, nb, n, f):
    import concourse.bass as bass
    from concourse import mybir
    from concourse.masks import make_identity
    from concourse.tile_rust import add_dep_helper

    nc = tc.nc
    f32 = mybir.dt.float32
    f32r = mybir.dt.float32r
    bf16 = mybir.dt.bfloat16
    nqt = n // P          # q tiles per batch (16)
    nkc = n // P          # key chunks for PV (16)
    W = n // 2            # psum half width (1024)
    GRP = 4               # q-tiles per PV group
    GW = GRP * P          # group width in q rows (512)
    Fa = f + 1            # V augmented with ones column
    NPAIR = nqt // 2      # tile pairs per batch (8)
    scale = 1.0 / np.sqrt(float(f))
    DCOLS = 640           # drain columns taken by DVE (rest on ACT)
    XCOLS = 192 # subtract columns taken by DVE (rest on Pool)

    _pend = []

    def absorb(*aps):
        return

    def dep(mm):
        for l in _pend:
            add_dep_helper(mm.ins, l.ins, sync=False, reason="wait-absorb")
        return mm

    def flush():
        _pend.clear()

    singles_cm = tc.tile_pool(name="singles", bufs=1)
    singles = singles_cm.__enter__()

    ident_f = singles.tile([P, P], f32)
    make_identity(nc, ident_f)
    ident_b = singles.tile([P, P], bf16)
    make_identity(nc, ident_b)

    wq_sb = singles.tile([f, f], f32)
    wk_sb = singles.tile([f, f], f32)
    wv_sb = singles.tile([f, f], f32)
    nc.sync.dma_start(out=wq_sb, in_=wq)
    nc.sync.dma_start(out=wk_sb, in_=wk)
    nc.sync.dma_start(out=wv_sb, in_=wv)
    wq_r = singles.tile([f, f], f32r)
    wk_r = singles.tile([f, f], f32r)
    wv_r = singles.tile([f, f], f32r)
    # fold the softmax scale AND the additive-mask 1/BIG into W_q
    nc.vector.tensor_scalar(out=wq_r, in0=wq_sb, scalar1=scale / BIG,
                            scalar2=None, op0=mybir.AluOpType.mult)
    nc.vector.tensor_copy(wk_r, wk_sb)
    nc.vector.tensor_copy(wv_r, wv_sb)

    # persistent per-batch tensors
    qt_sb = singles.tile([f, nb, n], f32r)   # Q^T per batch (pre-scaled)
    kt_sb = singles.tile([f, nb, n], f32r)
    v_sb = singles.tile([P, nb, nkc, Fa], bf16)  # V (+ones col) by key chunk

    # main-loop SBUF pools are allocated first so their addresses are
    # disjoint from the setup pools (avoids WAR waits on the first drains)
    t_p_cm = tc.tile_pool(name="t_p", bufs=6)
    u_p_cm = tc.tile_pool(name="u_p", bufs=4)
    eT_p_cm = tc.tile_pool(name="eT_p", bufs=2)
    small_cm = tc.tile_pool(name="small", bufs=8)
    res_p_cm = tc.tile_pool(name="res_p", bufs=2)
    t_p = t_p_cm.__enter__()
    u_p = u_p_cm.__enter__()
    eT_p = eT_p_cm.__enter__()
    small = small_cm.__enter__()
    res_p = res_p_cm.__enter__()

    # ---------------- setup: QKV (chunked) ----------------
    # x is loaded so partition p holds rows {t*512 + 4p + j : j<4} of the
    # batch (1KB contiguous elements).  The per-block transposes produce
    # columns in (j, p) order; the psum->sbuf copy writes them back in
    # (p, j) order so xT columns are the natural row order.  Setup psum
    # tiles borrow the main-loop pool tags (uT/o) since PSUM is fully
    # budgeted; batch 1's chunks are injected into early pipeline
    # iterations so the adjacency stream starts while QKV is still
    # being prepared.
    setup_sb_cm = tc.tile_pool(name="setup_sb", bufs=1)
    setup_sb = setup_sb_cm.__enter__()
    x_tiles = {}
    xT_tiles = {}

    s_ps_pool_cm = tc.tile_pool(name="s_ps", bufs=2, space="PSUM")
    uT_ps_pool_cm = tc.tile_pool(name="uT_ps", bufs=2, space="PSUM")
    o_ps_pool_cm = tc.tile_pool(name="o_ps", bufs=2, space="PSUM")
    s_ps_pool = s_ps_pool_cm.__enter__()
    uT_ps_pool = uT_ps_pool_cm.__enter__()
    o_ps_pool = o_ps_pool_cm.__enter__()

    def setup_x(b):
        # per-512-row-block loads so the first transpose only waits on a
        # quarter of the batch's x
        x_sb = setup_sb.tile([P, 4, 4, f], f32, tag="x")
        xv = x2[b].rearrange("(t p j) f -> p t (j f)", p=P, j=4)
        for t in range(4):
            nc.sync.dma_start(out=x_sb[:, t], in_=xv[:, t])
        x_tiles[b] = x_sb
        xT_sb = setup_sb.tile([f, 4, P, 4], f32r, tag="xT")
        xT_tiles[b] = xT_sb

    def setup_xt(b, t):
        x_sb = x_tiles[b]
        xT_sb = xT_tiles[b]
        xT_ps = uT_ps_pool.tile([f, 4, P], f32, tag="uT")
        for j in range(4):
            nc.tensor.transpose(xT_ps[:, j, :], x_sb[:, t, j, :], ident_f)
        # reorder (j, p) -> (p, j) while draining
        nc.scalar.copy(xT_sb[:, t].rearrange("f p j -> f j p"), xT_ps)

    def setup_qk(b, t, which):
        xT_sb = xT_tiles[b]
        xT_c = xT_sb[:, t].rearrange("f p j -> f (p j)")
        w_r, dst = ((wq_r, qt_sb), (wk_r, kt_sb))[which]
        qk_ps = uT_ps_pool.tile([f, 512], f32, tag="uT")
        nc.tensor.matmul(qk_ps, lhsT=w_r, rhs=xT_c, start=True, stop=True)
        nc.vector.tensor_copy(dst[:, b, t * 512:(t + 1) * 512], qk_ps)

    def setup_v(b, t):
        xT_sb = xT_tiles[b]
        xT_c = xT_sb[:, t].rearrange("f p j -> f (p j)")
        v_ps = o_ps_pool.tile([P, 4, f], f32, tag="o")
        for kc in range(4):
            nc.tensor.matmul(
                v_ps[:, kc, :],
                lhsT=xT_c[:, kc * P:(kc + 1) * P],
                rhs=wv_r, start=True, stop=True,
            )
        nc.scalar.activation(
            out=v_sb[:, b, 4 * t:4 * t + 4, 0:f], in_=v_ps,
            func=mybir.ActivationFunctionType.Copy, bias=0.0, scale=1.0)

    def setup_chunk(b, phase):
        # one phase per pipeline iteration for the deferred batch
        if phase == 0:
            for t in range(4):
                setup_xt(b, t)
        elif phase == 1:
            for t in range(4):
                setup_qk(b, t, 0)
        elif phase == 2:
            for t in range(4):
                setup_qk(b, t, 1)
        else:
            for t in range(4):
                setup_v(b, t)

    # softmax-denominator ones column (constant; must precede every PV)
    nc.vector.memset(v_sb[:, :, :, f:Fa], 1.0)
    setup_x(0)
    if nb > 1:
        setup_x(1)
    # warm the PE pstate while the x loads are in flight
    for wrm in range(12):
        warm_ps = uT_ps_pool.tile([P, P], bf16, tag="uT")
        nc.tensor.transpose(warm_ps, ident_b, ident_b)
    # minimal prefix for pair 0: all of K^T plus the first q chunk; the
    # rest of batch 0's QKV streams in behind the first S matmuls.
    for t in range(4):
        setup_xt(0, t)
    setup_qk(0, 0, 0)
    for t in range(4):
        setup_qk(0, t, 1)

    # ---------------- main loop ----------------
    if True:
        warm = small.tile([P, 1], f32, tag="dsc")
        nc.vector.memset(warm, 0.0)
        warm2 = small.tile([P, 1], f32, tag="dsc")
        nc.scalar.activation(out=warm2, in_=warm,
                             func=mybir.ActivationFunctionType.Exp)

        prev_exp = [None, None]   # last exp dest slice per half (ACT ticks)
        prev_t = [None, None]     # s_ps slot chase (per half)

        def front_tile(b, pair, ti, t2, dve_only=False):
            """S matmuls + split drain for one tile of the pair.  In the
            pipeline tail ACT is the bottleneck (it still owes the staggered
            exps), so the last pairs drain fully on DVE instead."""
            qi = 2 * pair + ti
            for h in range(2):
                s_ps = s_ps_pool.tile([P, W], f32, tag="s")
                for j in range(W // 512):
                    nc.tensor.matmul(
                        s_ps[:, j * 512:(j + 1) * 512],
                        lhsT=qt_sb[:, b, qi * P:(qi + 1) * P],
                        rhs=kt_sb[:, b,
                                  h * W + j * 512:h * W + (j + 1) * 512],
                        start=True, stop=True,
                    )
                if h == 0:
                    if dve_only:
                        nc.vector.tensor_copy(t2[:, ti, 0:W], s_ps)
                    else:
                        nc.vector.tensor_copy(
                            t2[:, ti, 0:DCOLS], s_ps[:, 0:DCOLS])
                        nc.scalar.activation(
                            out=t2[:, ti, DCOLS:W], in_=s_ps[:, DCOLS:W],
                            func=mybir.ActivationFunctionType.Copy,
                            bias=0.0, scale=1.0)
                else:
                    if dve_only:
                        nc.vector.tensor_copy(t2[:, ti, W:n], s_ps)
                    else:
                        nc.scalar.activation(
                            out=t2[:, ti, W:n], in_=s_ps,
                            func=mybir.ActivationFunctionType.Copy,
                            bias=0.0, scale=1.0)

        def emit_rmw_tile(b, pair, ti, t2):
            qi = 2 * pair + ti
            nc.gpsimd.dma_start(
                out=t2[:, ti, :],
                in_=adj2[b, qi * P:(qi + 1) * P, :],
                accum_op=mybir.AluOpType.add,
            )

        def emit_rmw(b, pair, t2):
            # masked-max mask: t1 = t0 + adj via casting RMW DMAs (SWDGE);
            # one DMA per tile so each rowmax waits only on its own half.
            for ti in range(2):
                emit_rmw_tile(b, pair, ti, t2)

        def emit_reduces(b, pair, t2):
            """negated masked row-max for both tiles of the pair."""
            nms = []
            for ti in range(2):
                negmax = small.tile([P, 1], f32, tag="m")
                nc.vector.tensor_reduce(
                    out=negmax, in_=t2[:, ti, :],
                    axis=mybir.AxisListType.X,
                    op=mybir.AluOpType.max, negate=True,
                )
                nms.append(negmax)
            return nms

        def back_sub(b, pair, t2, nms, tail=False):
            """u = (t1 + negmax) * BIG for both tiles (bf16, Pool + DVE).
            In the tail the two tiles go to DVE and Pool whole, in
            parallel, to shorten the drain-out critical chain."""
            uts = []
            for ti in range(2):
                u_t = u_p.tile([P, n], bf16, tag="u")
                xc = n if (tail and ti == 0) else (0 if tail else XCOLS)
                if xc:
                    nc.vector.tensor_scalar(
                        out=u_t[:, 0:xc], in0=t2[:, ti, 0:xc],
                        scalar1=nms[ti], scalar2=BIG,
                        op0=mybir.AluOpType.add, op1=mybir.AluOpType.mult,
                    )
                if xc < n:
                    nc.gpsimd.tensor_scalar(
                        out=u_t[:, xc:n], in0=t2[:, ti, xc:n],
                        scalar1=nms[ti], scalar2=BIG,
                        op0=mybir.AluOpType.add, op1=mybir.AluOpType.mult,
                    )
                uts.append(u_t)
            return uts

        def back_xe(b, pair, ti, u_t, eT_sb):
            """transpose + exp for one tile."""
            qi = 2 * pair + ti
            g = qi % GRP
            for hh in range(2):
                uT_ps = uT_ps_pool.tile([P, (nkc // 2) * P], bf16, tag="uT")
                for j8 in range(nkc // 2):
                    j = hh * (nkc // 2) + j8
                    nc.tensor.transpose(
                        uT_ps[:, j8 * P:(j8 + 1) * P],
                        u_t[:, j * P:(j + 1) * P],
                        ident_b,
                    )
                exp_dst = eT_sb[:, hh * (nkc // 2):(hh + 1) * (nkc // 2),
                                g * P:(g + 1) * P]
                nc.scalar.activation(
                    out=exp_dst,
                    in_=uT_ps.rearrange("p (j q) -> p j q", q=P),
                    func=mybir.ActivationFunctionType.Exp,
                )

        def pv_half(b, pair, eT_sb, oT_ps):
            """PV over this pair's 256 q-columns of the group."""
            c0 = (pair % 2) * 2 * P
            for j in range(nkc):
                nc.tensor.matmul(
                    oT_ps[:, c0:c0 + 2 * P],
                    lhsT=v_sb[:, b, j, :],
                    rhs=eT_sb[:, j, c0:c0 + 2 * P],
                    start=(j == 0), stop=(j == nkc - 1),
                )

        def finish_a(b, pair, oT_ps):
            """oT drain + strided transpose-back (group part 1)."""
            oT_sb = res_p.tile([Fa, GW], f32, tag="oT")
            nc.scalar.copy(oT_sb, oT_ps)
            oT_v = oT_sb.rearrange("f (p j) -> f j p", j=GRP)
            res4 = o_ps_pool.tile([P, GRP, Fa], f32, tag="o")
            for j in range(GRP):
                nc.tensor.transpose(
                    res4[:, j, :], oT_v[:, j, :], ident_f[0:Fa, 0:Fa],
                )
            return res4

        def finish_b(b, pair, res4):
            """1/Z scale + store (group part 2, one iteration later)."""
            qi = 2 * pair + 1
            r4 = small.tile([P, GRP], f32, tag="r4")
            nc.vector.reciprocal(r4, res4[:, :, f])
            res_sb = res_p.tile([P, GRP, f], f32, tag="res")
            for j in range(GRP):
                nc.vector.tensor_scalar(
                    out=res_sb[:, j, :], in0=res4[:, j, 0:f],
                    scalar1=r4[:, j:j + 1], scalar2=None,
                    op0=mybir.AluOpType.mult,
                )
            q0 = (qi - (GRP - 1)) * P
            nc.sync.dma_start(
                out=out2[b, q0:q0 + GW, :].rearrange(
                    "(p j) f -> p (j f)", p=P),
                in_=res_sb,
            )

        # Fine-grained 6-stage software pipeline (one iteration per tile
        # pair).  Stage offsets ensure every cross-engine dependency was
        # produced >= 1 iteration before an in-order engine queue reaches
        # its consumer, and PE always has ready work (transposes of i-4)
        # queued between the two S-matmul bursts of iteration i:
        #   i: S+drains | i-4: sub/transpose/exp | i-1: adjacency RMW |
        #   i-3: rowmaxes | i-5: half-PV (+ group finish on odd pairs)
        work = [(b, pair) for b in range(nb) for pair in range(NPAIR)]
        NW = len(work)
        tiles, maxes, eTs, oTs, us, res4s = {}, {}, {}, {}, {}, {}
        for i in range(NW + 6):
            if i < NW:
                b, pair = work[i]
                t2_new = t_p.tile([P, 2, n], f32, tag="t")
                tiles[i] = t2_new
                front_tile(b, pair, 0, tiles[i], dve_only=(i >= NW - 2))
                if i < 2:
                    emit_rmw_tile(b, pair, 0, tiles[i])
            if 3 <= i < NW + 1:
                b, pair = work[i - 1]
                emit_rmw(b, pair, tiles[i - 1])
            if 3 <= i < NW + 3:
                b, pair = work[i - 3]
                us[i - 3] = back_sub(b, pair, tiles[i - 3], maxes.pop(i - 3),
                                     tail=(i - 3 >= NW - 2))
            if i < NW:
                b, pair = work[i]
                front_tile(b, pair, 1, tiles[i], dve_only=(i >= NW - 2))
                if i < 2:
                    emit_rmw_tile(b, pair, 1, tiles[i])
            if 4 <= i < NW + 4:
                b, pair = work[i - 4]
                if pair % 2 == 0:
                    eT_new = eT_p.tile([P, nkc, GW], bf16, tag="eT")
                    eTs[i - 4] = eT_new
                else:
                    eTs[i - 4] = eTs[i - 5]
                back_xe(b, pair, 0, us[i - 4][0], eTs[i - 4])
                back_xe(b, pair, 1, us[i - 4][1], eTs[i - 4])
                us.pop(i - 4)
            if 2 <= i < NW + 2:
                b, pair = work[i - 2]
                maxes[i - 2] = emit_reduces(b, pair, tiles[i - 2])
            if 6 <= i < NW + 6 and (i - 6) in res4s:
                fb, fpair, fres4 = res4s.pop(i - 6)
                finish_b(fb, fpair, fres4)
            if 5 <= i < NW + 5:
                b, pair = work[i - 5]
                if pair % 2 == 0:
                    oT_new = o_ps_pool.tile([Fa, GW], f32, tag="o")
                    oTs[i - 5] = oT_new
                else:
                    oTs[i - 5] = oTs[i - 6]
                pv_half(b, pair, eTs[i - 5], oTs[i - 5])
                if pair % 2 == 1:
                    res4s[i - 5] = (b, pair, finish_a(b, pair, oTs[i - 5]))
                    for k in (i - 5, i - 6):
                        eTs.pop(k, None)
                        oTs.pop(k, None)
            if 5 <= i < NW + 5:
                tiles.pop(i - 5, None)
            if i == 0:
                for t in range(1, 4):
                    setup_qk(0, t, 0)
            elif i == 1:
                for t in range(4):
                    setup_v(0, t)
            elif 2 <= i <= 5 and nb > 1:
                setup_chunk(1, i - 2)

    for cm in (o_ps_pool_cm, uT_ps_pool_cm, s_ps_pool_cm, setup_sb_cm,
               res_p_cm, small_cm, eT_p_cm, u_p_cm, t_p_cm):
        cm.__exit__(None, None, None)
    singles_cm.__exit__(None, None, None)


def build_bass(nb=NB, n=N, f=F, num_devices=NCORES):
    import concourse.bass as bass
    import concourse.tile as tile
    from concourse import mybir

    nc = bass.Bass(
        "TRN2", target_bir_lowering=False, debug=False, num_devices=num_devices
    )
    x2 = nc.dram_tensor("x2", [nb, n, f], mybir.dt.float32,
                        kind="ExternalInput").ap()
    adj2 = nc.dram_tensor("adj2", [nb, n, n], mybir.dt.int32,
                          kind="ExternalInput").ap()
    wq = nc.dram_tensor("wq", [f, f], mybir.dt.float32, kind="ExternalInput").ap()
    wk = nc.dram_tensor("wk", [f, f], mybir.dt.float32, kind="ExternalInput").ap()
    wv = nc.dram_tensor("wv", [f, f], mybir.dt.float32, kind="ExternalInput").ap()
    out2 = nc.dram_tensor("out2", [nb, n, f], mybir.dt.float32,
                          kind="ExternalOutput").ap()
    with tile.TileContext(nc) as tc:
        build_kernel(tc, out2, x2, adj2, wq, wk, wv, nb=nb, n=n, f=f)
    return nc


_cached_nc = None


def kernel(x, adj, W_q, W_k, W_v, _trace=False):
    global _cached_nc
    _install_compile_patch()
    from concourse import bass_utils

    if _cached_nc is None:
        _cached_nc = build_bass()
    nc = _cached_nc

    x = np.ascontiguousarray(np.asarray(x, dtype=np.float32))
    adj = np.ascontiguousarray(np.asarray(adj, dtype=np.int32))
    wq = np.ascontiguousarray(np.asarray(W_q, dtype=np.float32))
    wk = np.ascontiguousarray(np.asarray(W_k, dtype=np.float32))
    wv = np.ascontiguousarray(np.asarray(W_v, dtype=np.float32))

    in_maps = []
    for c in range(NCORES):
        in_maps.append({
            "x2": x[c * NB:(c + 1) * NB],
            "adj2": adj[c * NB:(c + 1) * NB],
            "wq": wq, "wk": wk, "wv": wv,
        })
    res = bass_utils.run_bass_kernel_spmd(
        nc, in_maps, core_ids=list(range(NCORES)), trace=_trace,
    )
    out = np.concatenate([r["out2"] for r in res.results], axis=0)
    if _trace:
        kernel._last_results = res
    return out.reshape(B, N, F)
